# revision 1
# baseline (speedup 1.0000x reference)
"""Trainium2 Bass kernel for nn_DTFOS: fractional differencing residual.

Per batch b (one per NeuronCore, 8 cores):
    Y = fracdiff(X, relu(alpha))      # causal conv with (1-L)^alpha weights
    E = Y[1:, :] - X[:-1, :] @ A.T

Algorithm per core (128 channels):
  - Build w[k] on device: log-domain cumsum (tensor_tensor_scan) + exp.
  - Length-16384 FFT convolution per channel, radix-128 x 128 two-stage DFT:
      stage1 (contract a, PE matmul vs DFT-128 consts) -> DRAM bounce
      transpose -> twiddle (DVE, bf16) -> stage2 (PE) -> spectrum product
      (DVE) -> inverse stage (PE) -> inv twiddle -> DRAM bounce transpose ->
      final inverse (PE, real part only)
  - Yhat = X @ A^T via PE (per-block transposes), subtracted at the end.

kernel(**inputs) takes FULL inputs (8, 8192, 128)/(8, 128)/(8, 128, 128),
shards batch over 8 cores, returns FULL output (8, 8191, 128) fp32.
"""
import sys
import os
import numpy as np

sys.path.insert(0, "/opt/trn_rl_repo")

import ml_dtypes  # noqa: E402
from contextlib import ExitStack  # noqa: E402

import concourse.bass as bass  # noqa: E402
import concourse.mybir as mybir  # noqa: E402
import concourse.tile as tile  # noqa: E402
from concourse.masks import make_identity  # noqa: E402

F32 = mybir.dt.float32
F32R = mybir.dt.float32r
BF16 = mybir.dt.bfloat16
AF = mybir.ActivationFunctionType
OP = mybir.AluOpType

T = 8192          # time steps
NCH = 128         # channels per core
L = 16384         # FFT length
N = 128           # radix (both factors)
A64 = 64          # nonzero a-rows after zero padding
CH = 16           # channels per pipeline chunk
NCHUNK = NCH // CH
MMF = 512         # matmul moving free size
SPECT = BF16      # spectrum dtype


def _host_consts():
    a = np.arange(A64, dtype=np.float64)[:, None]
    c = np.arange(N, dtype=np.float64)[None, :]
    ph1 = 2.0 * np.pi * a * c / N
    consts = {}
    consts["F1R"] = np.cos(ph1).astype(np.float32)          # [a, c]
    consts["F1I"] = (-np.sin(ph1)).astype(np.float32)

    b = np.arange(N, dtype=np.float64)[:, None]
    d = np.arange(N, dtype=np.float64)[None, :]
    ph2 = 2.0 * np.pi * b * d / N
    bf = ml_dtypes.bfloat16
    consts["E2R"] = np.cos(ph2).astype(bf)                   # [b, d]
    consts["E2I"] = (-np.sin(ph2)).astype(bf)
    consts["E2NI"] = (np.sin(ph2)).astype(bf)
    consts["G2R"] = np.cos(ph2).astype(bf)                   # [d, b']
    consts["G2I"] = (np.sin(ph2)).astype(bf)
    consts["G2NI"] = (-np.sin(ph2)).astype(bf)
    ap = np.arange(A64, dtype=np.float64)[None, :]
    cp = np.arange(N, dtype=np.float64)[:, None]
    ph3 = 2.0 * np.pi * cp * ap / N
    consts["H1R"] = (np.cos(ph3) / L).astype(bf)             # [c, a']
    consts["H1NI"] = (-np.sin(ph3) / L).astype(bf)

    # twiddles, ch-broadcast, transposed layouts
    bb = np.arange(N, dtype=np.float64)[:, None]
    cc = np.arange(N, dtype=np.float64)[None, :]
    phT = 2.0 * np.pi * bb * cc / L
    twfr = np.cos(phT)
    twfi = -np.sin(phT)
    consts["TWFR"] = np.repeat(twfr[:, :, None], CH, axis=2).reshape(N, N * CH).astype(bf)
    consts["TWFI"] = np.repeat(twfi[:, :, None], CH, axis=2).reshape(N, N * CH).astype(bf)
    twir = np.cos(phT)
    twii = np.sin(phT)
    consts["TWIR"] = np.repeat(twir[:, :, None], CH, axis=2).reshape(N, N * CH).astype(bf)
    consts["TWII"] = np.repeat(twii[:, :, None], CH, axis=2).reshape(N, N * CH).astype(bf)

    # w-construction tables
    k = np.arange(T, dtype=np.float64)
    kt = k - 1.0
    kt[0] = 2.0
    kt[1] = 2.0
    consts["KT"] = kt.astype(np.float32)[None, :]            # [1, T]
    lnk = np.zeros(T)
    lnk[2:] = np.cumsum(np.log(k[2:]))
    consts["CT"] = lnk.astype(np.float32)[None, :]           # [1, T]
    return consts


_CONSTS = _host_consts()


def build_program():
    nc = bass.Bass()
    x_h = nc.declare_dram_parameter("X", [T, NCH], F32, isOutput=False)
    al_h = nc.declare_dram_parameter("alpha", [NCH, 1], F32, isOutput=False)
    a_h = nc.declare_dram_parameter("A", [NCH, NCH], F32, isOutput=False)
    ch_: dict[str, bass.AP] = {}
    for name, arr in _CONSTS.items():
        dt = F32 if arr.dtype == np.float32 else BF16
        ch_[name] = nc.declare_dram_parameter(name, list(arr.shape), dt, isOutput=False)
    e_h = nc.declare_dram_parameter("E", [T - 1, NCH], F32, isOutput=True)

    # DRAM scratch
    wd_h = nc.dram_tensor("wd", [T, NCH], F32R)              # w in X-layout (f32r)
    xr_h = nc.dram_tensor("Xr", [T, NCH], F32R)              # rounded X
    yh_h = nc.dram_tensor("YH", [T, NCH], F32)               # Yhat rows
    yb_h = {}
    for q in range(NCHUNK):
        for nm in ("xr", "xi", "wr", "wi"):
            yb_h[(q, nm)] = nc.dram_tensor(f"YB_{q}_{nm}", [N, N, CH], SPECT)
        for nm in ("ur", "ui"):
            yb_h[(q, nm)] = nc.dram_tensor(f"UB_{q}_{nm}", [N, N, CH], SPECT)

    hw = nc.hwdge_engines
    dmae = [getattr(nc, e.name.lower(), None) for e in hw] if hw else [nc.sync]
    dmae = [e for e in dmae if e is not None] or [nc.sync]

    def dma(i, out, in_):
        eng = dmae[i % len(dmae)]
        with nc.allow_non_contiguous_dma(reason="layout"):
            eng.dma_start(out=out, in_=in_)

    with tile.TileContext(nc) as tc, ExitStack() as ctx:
        consts = ctx.enter_context(tc.tile_pool(name="consts", bufs=1))
        # ---- load constants ----
        cs = {}
        for name in ("F1R", "F1I"):
            cs[name] = consts.tile([A64, N], F32, tag=name, name=name)
            nc.sync.dma_start(out=cs[name], in_=ch_[name][:])
        for name in ("E2R", "E2I", "E2NI", "G2R", "G2I", "G2NI"):
            cs[name] = consts.tile([N, N], BF16, tag=name, name=name)
            nc.sync.dma_start(out=cs[name], in_=ch_[name][:])
        for name in ("H1R", "H1NI"):
            cs[name] = consts.tile([N, A64], BF16, tag=name, name=name)
            nc.sync.dma_start(out=cs[name], in_=ch_[name][:])
        for name in ("TWFR", "TWFI", "TWIR", "TWII"):
            cs[name] = consts.tile([N, N, CH], BF16, tag=name, name=name)
            nc.sync.dma_start(out=cs[name], in_=ch_[name][:].rearrange("b (c h) -> b c h", h=CH))
        for name in ("F1R", "F1I"):
            rname = name + "r"
            cs[rname] = consts.tile([A64, N], F32R, tag=rname, name=rname)
            nc.scalar.activation(cs[rname][:], cs[name][:], AF.Copy)
        ident = consts.tile([N, N], F32, tag="ident")
        make_identity(nc, ident[:])
        alr0 = consts.tile([NCH, 1], F32, tag="alr0")
        nc.sync.dma_start(out=alr0, in_=al_h[:])
        alr = consts.tile([NCH, 1], F32, tag="alr")
        nc.vector.tensor_copy(alr[:], alr0[:])
        nc.vector.tensor_scalar_max(alr[:], alr[:], 0.0)
        lga = consts.tile([NCH, 1], F32, tag="lga")
        nc.scalar.activation(lga[:], alr[:], AF.Ln)
        lgav = consts.tile([NCH, 1], F32, tag="lgav")
        nc.vector.tensor_copy(lgav[:], lga[:])

        early = ExitStack()
        pt128 = early.enter_context(tc.tile_pool(name="pt128", bufs=2, space="PSUM"))

        # ================= phase W: build w, write wd (X-layout) ============
        with tc.tile_pool(name="wph", bufs=1) as wph, \
             tc.tile_pool(name="wch_p", bufs=1) as wch_p:
            wch = wch_p.tile([NCH, T], F32, tag="wch")
            H = T // 2
            cum = wph.tile([NCH, T], F32, tag="cum", name="cum")
            for h in range(2):
                sl = slice(h * H, (h + 1) * H)
                ktb = wph.tile([NCH, H], F32, tag="ktb", bufs=2)
                ctb = wph.tile([NCH, H], F32, tag="ctb", bufs=2)
                dma(0, ktb[:], ch_["KT"][:, sl].to_broadcast([NCH, H]))
                dma(1, ctb[:], ch_["CT"][:, sl].to_broadcast([NCH, H]))
                t1 = wph.tile([NCH, H], F32, tag="t1")
                nc.vector.tensor_copy(t1[:], ktb[:])
                nc.vector.tensor_scalar(out=t1[:], in0=t1[:], scalar1=alr[:],
                                        scalar2=None, op0=OP.subtract)
                nc.scalar.activation(t1[:], t1[:], AF.Ln)  # ln(k-1-alpha)
                if h == 0:
                    nc.vector.memset(t1[:, 0:2], 0.0)
                nc.vector.tensor_tensor_scan(out=cum[:, sl], data0=t1[:], data1=t1[:],
                                             initial=0.0, op0=OP.add, op1=OP.bypass)
                if h == 0:
                    bias = lgav
                else:
                    bias = wph.tile([NCH, 1], F32, tag="bias", name="bias")
                    nc.vector.tensor_add(bias[:], lgav[:], cum[:, H - 1:H])
                lw = wph.tile([NCH, H], F32, tag="lw")
                # lw = (cum - lnk_cumsum) + (ln(alpha) [+ prev half total])
                nc.vector.tensor_sub(lw[:], cum[:, sl], ctb[:])
                nc.vector.tensor_scalar(out=lw[:], in0=lw[:], scalar1=bias[:],
                                        scalar2=None, op0=OP.add)
                nc.scalar.activation(wch[:, sl], lw[:], AF.Exp)
            negone = wch_p.tile([NCH, 1], F32, tag="negone", name="negone")
            nc.vector.memset(negone[:], -1.0)
            nc.vector.tensor_tensor(out=wch[:], in0=wch[:],
                                    in1=negone[:].to_broadcast([NCH, T]),
                                    op=OP.mult)
            nc.vector.memset(wch[:, 0:1], 1.0)
            # transpose to X-layout in DRAM: wd[128a+b, ch] = wch[ch, 128a+b]
            with tc.tile_pool(name="wtr", bufs=3) as wtr:
                for a in range(A64):
                    pt = pt128.tile([N, N], F32, tag="ptw")
                    nc.tensor.transpose(pt[:], wch[:, a * N:(a + 1) * N], ident[:])
                    sb = wtr.tile([N, N], F32R, tag="wtsb")
                    nc.scalar.activation(sb[:], pt[:], AF.Copy)
                    dma(a, wd_h[a * N:(a + 1) * N, :], sb[:])

        tc.strict_bb_all_engine_barrier()
        # ================= phase Yhat: X @ A^T -> YH dram ===================
        with tc.tile_pool(name="bmm", bufs=3) as bmm:
            an = bmm.tile([N, N], F32, tag="an")
            nc.sync.dma_start(out=an, in_=a_h[:])
            pa = pt128.tile([N, N], F32, tag="ptw")
            nc.tensor.transpose(pa[:], an[:], ident[:])
            at = consts.tile([N, N], F32, tag="at")
            nc.scalar.activation(at[:], pa[:], AF.Copy)
            for blk in range(A64):
                xn = bmm.tile([N, N], F32, tag="xn", bufs=6)
                dma(blk, xn[:], x_h[blk * N:(blk + 1) * N, :])
                px = pt128.tile([N, N], F32, tag="ptw")
                nc.tensor.transpose(px[:], xn[:], ident[:])
                xt = bmm.tile([N, N], F32, tag="xt")
                nc.scalar.activation(xt[:], px[:], AF.Copy)
                xrr = bmm.tile([N, N], F32R, tag="xrr")
                nc.scalar.activation(xrr[:], xn[:], AF.Copy)
                dma(blk, xr_h[blk * N:(blk + 1) * N, :], xrr[:])
                pb = pt128.tile([N, N], F32, tag="ptb")
                nc.tensor.matmul(pb[:], xt[:], at[:], start=True, stop=True)
                yh = bmm.tile([N, N], F32, tag="yh")
                nc.scalar.activation(yh[:], pb[:], AF.Copy)
                dma(blk, yh_h[blk * N:(blk + 1) * N, :], yh[:])

        tc.strict_bb_all_engine_barrier()
        early.close()

        # ================= FFT conv pipeline, per channel chunk =============
        xv = xr_h[:].rearrange("(a b) c -> a b c", b=N)      # [64, 128, 128]
        wv = wd_h[:].rearrange("(a b) c -> a b c", b=N)
        yhv = yh_h[:].rearrange("(a b) c -> a b c", b=N)

        mov = ctx.enter_context(tc.tile_pool(name="mov", bufs=2))
        spec = ctx.enter_context(tc.tile_pool(name="spec", bufs=1))
        ps1 = ctx.enter_context(tc.tile_pool(name="ps1", bufs=4, space="PSUM"))
        psy = ctx.enter_context(tc.tile_pool(name="psy", bufs=2, space="PSUM"))
        NS = (N * CH) // MMF                                  # 512-slices per pass

        for q in range(NCHUNK):
            c0 = q * CH
            # ---- stage 1 (contract a): Y[c, (b ch)] -> bounce to DRAM ----
            for nm, src in (("x", xv), ("w", wv)):
                mv = mov.tile([A64, N, CH], F32R, tag="mv")
                dma(q, mv[:], src[:, :, c0:c0 + CH])
                for comp, st in (("r", "F1R"), ("i", "F1I")):
                    yo = spec.tile([N, N, CH], SPECT, tag="yo", bufs=2)
                    for s in range(NS):
                        ps = ps1.tile([N, MMF], F32, tag="ps1t")
                        w0 = s * MMF // CH                    # b-offset of slice
                        bw = MMF // CH
                        nc.tensor.matmul(
                            ps[:],
                            cs[st + "r"][:],
                            mv[:, w0:w0 + bw, :],
                            start=True, stop=True)
                        nc.scalar.activation(
                            yo[:, w0:w0 + bw, :],
                            ps[:].rearrange("c (b h) -> c b h", h=CH), AF.Copy)
                    dma(q, yb_h[(q, nm + comp)][:], yo[:])

            # ---- bounce back transposed + twiddle + stage 2 ----
            sS = {}
            for nm in ("x", "w"):
                ytr = spec.tile([N, N, CH], SPECT, tag="ytr", bufs=2)  # [b, c, ch]
                yti = spec.tile([N, N, CH], SPECT, tag="yti", bufs=2)
                dma(q, ytr[:], yb_h[(q, nm + "r")][:].transpose([1, 0, 2]))
                dma(q + 1, yti[:], yb_h[(q, nm + "i")][:].transpose([1, 0, 2]))
                m1 = spec.tile([N, N, CH], SPECT, tag="m1")
                m2 = spec.tile([N, N, CH], SPECT, tag="m2")
                zr = spec.tile([N, N, CH], SPECT, tag="zr")
                zi = spec.tile([N, N, CH], SPECT, tag="zi")
                nc.vector.tensor_mul(m1[:], ytr[:], cs["TWFR"][:])
                nc.vector.tensor_mul(m2[:], yti[:], cs["TWFI"][:])
                nc.vector.tensor_sub(zr[:], m1[:], m2[:])
                nc.vector.tensor_mul(m1[:], ytr[:], cs["TWFI"][:])
                nc.vector.tensor_mul(m2[:], yti[:], cs["TWFR"][:])
                nc.vector.tensor_add(zi[:], m1[:], m2[:])
                # stage 2: contract b
                sr = spec.tile([N, N, CH], SPECT, tag="sr" + nm)
                si = spec.tile([N, N, CH], SPECT, tag="si" + nm)
                for s in range(NS):
                    w0 = s * MMF // CH
                    bw = MMF // CH
                    pr = ps1.tile([N, MMF], F32, tag="ps1t")
                    nc.tensor.matmul(pr[:], cs["E2R"][:], zr[:, w0:w0 + bw, :],
                                     start=True, stop=False)
                    nc.tensor.matmul(pr[:], cs["E2NI"][:], zi[:, w0:w0 + bw, :],
                                     start=False, stop=True)
                    nc.scalar.activation(sr[:, w0:w0 + bw, :],
                                          pr[:].rearrange("d (c h) -> d c h", h=CH),
                                          AF.Copy)
                    pi = ps1.tile([N, MMF], F32, tag="ps1t")
                    nc.tensor.matmul(pi[:], cs["E2I"][:], zr[:, w0:w0 + bw, :],
                                     start=True, stop=False)
                    nc.tensor.matmul(pi[:], cs["E2R"][:], zi[:, w0:w0 + bw, :],
                                     start=False, stop=True)
                    nc.scalar.activation(si[:, w0:w0 + bw, :],
                                          pi[:].rearrange("d (c h) -> d c h", h=CH),
                                          AF.Copy)
                sS[nm] = (sr, si)

            # ---- product ----
            (sxr, sxi), (swr, swi) = sS["x"], sS["w"]
            m1 = spec.tile([N, N, CH], SPECT, tag="m1")
            m2 = spec.tile([N, N, CH], SPECT, tag="m2")
            ppr = spec.tile([N, N, CH], SPECT, tag="ppr")
            ppi = spec.tile([N, N, CH], SPECT, tag="ppi")
            nc.vector.tensor_mul(m1[:], sxr[:], swr[:])
            nc.vector.tensor_mul(m2[:], sxi[:], swi[:])
            nc.vector.tensor_sub(ppr[:], m1[:], m2[:])
            nc.vector.tensor_mul(m1[:], sxr[:], swi[:])
            nc.vector.tensor_mul(m2[:], sxi[:], swr[:])
            nc.vector.tensor_add(ppi[:], m1[:], m2[:])

            # ---- inverse stage A (contract d) + inv twiddle ----
            ur = spec.tile([N, N, CH], SPECT, tag="ur")
            ui = spec.tile([N, N, CH], SPECT, tag="ui")
            for s in range(NS):
                w0 = s * MMF // CH
                bw = MMF // CH
                pr = ps1.tile([N, MMF], F32, tag="ps1t")
                nc.tensor.matmul(pr[:], cs["G2R"][:], ppr[:, w0:w0 + bw, :],
                                 start=True, stop=False)
                nc.tensor.matmul(pr[:], cs["G2NI"][:], ppi[:, w0:w0 + bw, :],
                                 start=False, stop=True)
                nc.scalar.activation(ur[:, w0:w0 + bw, :],
                                      pr[:].rearrange("b (c h) -> b c h", h=CH),
                                      AF.Copy)
                pi = ps1.tile([N, MMF], F32, tag="ps1t")
                nc.tensor.matmul(pi[:], cs["G2I"][:], ppr[:, w0:w0 + bw, :],
                                 start=True, stop=False)
                nc.tensor.matmul(pi[:], cs["G2R"][:], ppi[:, w0:w0 + bw, :],
                                 start=False, stop=True)
                nc.scalar.activation(ui[:, w0:w0 + bw, :],
                                      pi[:].rearrange("b (c h) -> b c h", h=CH),
                                      AF.Copy)
            m1 = spec.tile([N, N, CH], SPECT, tag="m1")
            m2 = spec.tile([N, N, CH], SPECT, tag="m2")
            upr = spec.tile([N, N, CH], SPECT, tag="upr")
            upi = spec.tile([N, N, CH], SPECT, tag="upi")
            nc.vector.tensor_mul(m1[:], ur[:], cs["TWIR"][:])
            nc.vector.tensor_mul(m2[:], ui[:], cs["TWII"][:])
            nc.vector.tensor_sub(upr[:], m1[:], m2[:])
            nc.vector.tensor_mul(m1[:], ur[:], cs["TWII"][:])
            nc.vector.tensor_mul(m2[:], ui[:], cs["TWIR"][:])
            nc.vector.tensor_add(upi[:], m1[:], m2[:])

            # ---- bounce 2 + inverse stage B (contract c, real out) ----
            dma(q, yb_h[(q, "ur")][:], upr[:])
            dma(q + 1, yb_h[(q, "ui")][:], upi[:])
            utr = spec.tile([N, N, CH], SPECT, tag="utr", bufs=2)     # [c, b', ch]
            uti = spec.tile([N, N, CH], SPECT, tag="uti", bufs=2)
            dma(q, utr[:], yb_h[(q, "ur")][:].transpose([1, 0, 2]))
            dma(q + 1, uti[:], yb_h[(q, "ui")][:].transpose([1, 0, 2]))
            yf = spec.tile([A64, N, CH], F32, tag="yf")       # conv result
            for s in range(NS):
                w0 = s * MMF // CH
                bw = MMF // CH
                py = psy.tile([A64, MMF], F32, tag="psyt")
                nc.tensor.matmul(py[:], cs["H1R"][:], utr[:, w0:w0 + bw, :],
                                 start=True, stop=False)
                nc.tensor.matmul(py[:], cs["H1NI"][:], uti[:, w0:w0 + bw, :],
                                 start=False, stop=True)
                nc.scalar.activation(yf[:, w0:w0 + bw, :],
                                      py[:].rearrange("a (b h) -> a b h", h=CH),
                                      AF.Copy)

            # ---- E = y[t+1] - Yhat[t]; write out ----
            yh = spec.tile([A64, N, CH], F32, tag="yhc", bufs=2)
            dma(q, yh[:], yhv[:, :, c0:c0 + CH])
            ee = spec.tile([A64, N, CH], F32, tag="ee")
            nc.vector.tensor_sub(ee[:, 0:N - 1, :], yf[:, 1:N, :], yh[:, 0:N - 1, :])
            ytail = spec.tile([A64 - 1, 1, CH], F32, tag="ytail")
            dma(q, ytail[:], yf[1:A64, 0:1, :])
            nc.vector.tensor_sub(ee[0:A64 - 1, N - 1:N, :], ytail[:],
                                 yh[0:A64 - 1, N - 1:N, :])
            ev = e_h[0:(A64 - 1) * N, :].rearrange("(a b) c -> a b c", b=N)
            dma(q, ev[:, :, c0:c0 + CH], ee[0:A64 - 1, :, :])
            dma(q + 1, e_h[(A64 - 1) * N:T - 1, c0:c0 + CH],
                ee[A64 - 1:A64, 0:N - 1, :])

    _split_waits(nc)
    return nc


def _split_waits(nc):
    """Walrus allows 1 inline sem-wait per compute instruction (2 per DMA).
    Hoist excess waits into standalone EventSemaphore instructions on the
    same engine right before the instruction (semantically identical)."""
    caps = {}
    n_split = 0
    for fn in nc.m.functions:
        for blk in fn.blocks:
            out = []
            for ins in blk.instructions:
                si = getattr(ins, "sync_info", None)
                waits = list(si.on_wait) if si is not None and si.on_wait else []
                cap = caps.get(str(ins.opcode), 1)
                if len(waits) > cap:
                    for k, w in enumerate(waits[:-cap]):
                        es = mybir.InstEventSemaphore(
                            name=f"wsp_{ins.name}_{k}")
                        es.engine = ins.engine
                        es.sync_info = mybir.SyncInfo(on_wait=[w], on_update=[])
                        out.append(es)
                        n_split += 1
                    si.on_wait = waits[-cap:]
                out.append(ins)
            blk.instructions = out
    return n_split


_NC = None


def _get_nc():
    global _NC
    if _NC is None:
        _NC = build_program()
    return _NC


def kernel(X, alpha, A):
    from concourse.bass_utils import run_bass_kernel_spmd
    nc = _get_nc()
    B = X.shape[0]
    core_ids = list(range(B))
    in_maps = []
    for b in range(B):
        m = {"X": np.ascontiguousarray(X[b], dtype=np.float32),
             "alpha": np.ascontiguousarray(alpha[b].reshape(NCH, 1), dtype=np.float32),
             "A": np.ascontiguousarray(A[b], dtype=np.float32)}
        for name, arr in _CONSTS.items():
            m[name] = arr
        in_maps.append(m)
    res = run_bass_kernel_spmd(nc, in_maps, core_ids)
    out = np.stack([res.results[b]["E"] for b in range(B)], axis=0)
    return out.astype(np.float32)

